# revision 1
# baseline (speedup 1.0000x reference)
"""Block sliding-window attention on 8 TRN2 NeuronCores.

Sharding: sequence-parallel. 8 shards = (batch b in {0,1}) x (quarter s in
0..3); each core owns 2048 consecutive tokens of one batch plus a 256-token
K/V halo from the previous quarter (zeros + -inf gate for the first quarter).
No collectives: each core computes its tokens' full output rows.

Per-core pipeline (all matmuls in float32r: full PE rate, ~1e-4 rounding):
  P1: QT/KT = W^T @ hiddenT (head-transposed layout, raw), V = hidden @ Wv
      (natural layout), all staged through DRAM scratch.
  P2: per 256-token chunk: RoPE on Q/K (rot-half via partition-offset DMA
      reload + pre-signed sin), then per head: S^T = K Q^T per 128-key block,
      exp on ACT (scale=1/sqrt(128), -1e30 bias gates the no-previous case),
      0/1 triangular mask multiply on DVE (also retypes to f32r), denominator
      via all-ones matmul (broadcasts across partitions), O^T = V^T P^T,
      normalize with DVE reciprocal.
  P3: out = sum_h O_h @ Wo_h, accumulated over all 16 head blocks in PSUM.
"""
import sys

try:
    import concourse  # noqa: F401
except ImportError:
    sys.path.insert(0, '/opt/trn_rl_repo')

import ml_dtypes
import numpy as np

import concourse.bacc as bacc
import concourse.mybir as mybir
import concourse.tile as tile
from concourse.bass_utils import run_bass_kernel_spmd

f32 = mybir.dt.float32
f32r = mybir.dt.float32r
AF = mybir.ActivationFunctionType
bf16 = mybir.dt.bfloat16

DIMS = 2048
HEADS = 16
HD = 128           # head dim
WIN = 256          # window / chunk
B, S = 2, 8192
NSH = 4            # seq shards per batch
THETA = 10000.0
ISQ = float(1.0 / np.sqrt(HD))
IB = DIMS // 128   # 16 input-dim blocks


def build(nc, T, phases=(1, 2, 3)):
    """Emit the per-core program. T = local tokens (multiple of 512)."""
    TH = T + WIN                      # with halo
    NC_ = T // WIN                    # chunks
    HT = nc.dram_tensor("HT", [DIMS, TH], f32r, kind="ExternalInput")
    WQ = nc.dram_tensor("WQ", [DIMS, DIMS], f32r, kind="ExternalInput")
    WK = nc.dram_tensor("WK", [DIMS, DIMS], f32r, kind="ExternalInput")
    WV = nc.dram_tensor("WV", [DIMS, DIMS], f32r, kind="ExternalInput")
    WO = nc.dram_tensor("WO", [DIMS, DIMS], f32r, kind="ExternalInput")
    COS = nc.dram_tensor("COS", [HD, TH], f32, kind="ExternalInput")
    SINS = nc.dram_tensor("SINS", [HD, TH], f32, kind="ExternalInput")
    TRI23 = nc.dram_tensor("TRI23", [128, 2 * WIN], bf16, kind="ExternalInput")
    PGATE = nc.dram_tensor("PGATE", [128, 1], f32, kind="ExternalInput")
    ONESM = nc.dram_tensor("ONESM", [128, 128], bf16, kind="ExternalInput")
    OUT = nc.dram_tensor("OUT", [T, DIMS], f32, kind="ExternalOutput")

    QTS = nc.dram_tensor("QTS", [HEADS, HD, T], bf16)    # raw (pre-RoPE) Q^T
    KTS = nc.dram_tensor("KTS", [HEADS, HD, TH], bf16)   # raw K^T (with halo)
    VS = nc.dram_tensor("VS", [TH, DIMS], bf16)         # V natural
    OTS = nc.dram_tensor("OTS", [HEADS, HD, T], f32r)   # normalized O^T

    def tok_tiles(n):
        out, a = [], 0
        while a < n:
            w = min(512, n - a)
            out.append((a, w))
            a += w
        return out

    with tile.TileContext(nc) as tc:
        with tc.tile_pool(name="cst", bufs=1) as cst:
            tri23 = cst.tile([128, 2 * WIN], bf16)
            pgate = cst.tile([128, 1], f32)
            onesm = cst.tile([128, 128], bf16)
            nc.sync.dma_start(tri23[:], TRI23[:])
            nc.sync.dma_start(pgate[:], PGATE[:])
            nc.sync.dma_start(onesm[:], ONESM[:])

            # ---------------- P1: projections ----------------
            if 1 in phases:
              with tc.tile_pool(name="p1", bufs=1) as p1, \
                 tc.tile_pool(name="wp", bufs=10) as wp, \
                 tc.tile_pool(name="st", bufs=8) as st, \
                 tc.tile_pool(name="pp", bufs=8, space="PSUM") as pp:
                ht = p1.tile([128, IB, TH], f32r)
                nc.sync.dma_start(ht[:], HT.rearrange("(ib p) t -> p ib t", p=128))

                # QT / KT: lhsT = W block [128in, 128out], rhs = hT
                for W_, DST, t0, tlen in ((WQ, QTS, WIN, T), (WK, KTS, 0, TH)):
                    for ob in range(HEADS):
                        tts = tok_tiles(tlen)
                        psums = [pp.tile([128, 512], f32, name="pp") for _ in tts]
                        for ib in range(IB):
                            wt = wp.tile([128, 128], f32r, name="w")
                            nc.sync.dma_start(
                                wt[:], W_[ib * 128:(ib + 1) * 128,
                                          ob * 128:(ob + 1) * 128])
                            for ti, (a, w) in enumerate(tts):
                                nc.tensor.matmul(
                                    psums[ti][:, :w], wt[:],
                                    ht[:, ib, t0 + a:t0 + a + w],
                                    start=(ib == 0), stop=(ib == IB - 1))
                        for ti, (a, w) in enumerate(tts):
                            so = st.tile([128, 512], bf16, name="st")
                            nc.scalar.copy(so[:, :w], psums[ti][:, :w])
                            nc.sync.dma_start(DST[ob][:, a:a + w], so[:, :w])

                # V natural: lhsT = hT block [128in, 128tok], rhs = Wv rows
                NTB = TH // 128
                for tb0 in range(0, NTB, 6):
                    tbs = list(range(tb0, min(tb0 + 6, NTB)))
                    for og in range(4):
                        psums = {}
                        for ib in range(IB):
                            wt = wp.tile([128, 512], f32r, name="wv")
                            nc.sync.dma_start(
                                wt[:], WV[ib * 128:(ib + 1) * 128,
                                          og * 512:(og + 1) * 512])
                            for tb in tbs:
                                if ib == 0:
                                    psums[tb] = pp.tile([128, 512], f32, name="pp")
                                nc.tensor.matmul(
                                    psums[tb][:],
                                    ht[:, ib, tb * 128:(tb + 1) * 128], wt[:],
                                    start=(ib == 0), stop=(ib == IB - 1))
                        for tb in tbs:
                            so = st.tile([128, 512], bf16, name="stv")
                            nc.vector.tensor_copy(so[:], psums[tb][:])
                            nc.sync.dma_start(
                                VS[tb * 128:(tb + 1) * 128,
                                   og * 512:(og + 1) * 512], so[:])

            # ---------------- P2: attention ----------------
            if 2 in phases:
              with tc.tile_pool(name="qk", bufs=2) as qk, \
                 tc.tile_pool(name="rt", bufs=1) as rt, \
                 tc.tile_pool(name="tp", bufs=3) as tp, \
                 tc.tile_pool(name="ptp", bufs=2) as ptp, \
                 tc.tile_pool(name="ex", bufs=2) as exp_pool, \
                 tc.tile_pool(name="ob", bufs=2) as obp, \
                 tc.tile_pool(name="ps_s", bufs=4, space="PSUM") as ps_s, \
                 tc.tile_pool(name="ps_d", bufs=2, space="PSUM") as ps_d, \
                 tc.tile_pool(name="ps_o", bufs=2, space="PSUM") as ps_o:
                def rope_load(SRC, c0, roped, which, pos0=None):
                    """Load [128, HEADS, WIN] token window at c0 from SRC
                    (head-major scratch), apply RoPE into `roped` (f32r).
                    pos0: column into COS/SINS (halo coords); default c0.
                    cos/sin slices are DMA-replicated x4 so the DVE ops run
                    on [128, 4*WIN] four-head groups."""
                    if pos0 is None:
                        pos0 = c0
                    raw = rt.tile([128, HEADS, WIN], bf16, name=f"raw{which}")
                    rot = rt.tile([128, HEADS, WIN], bf16, name=f"rot{which}")
                    sl = SRC[:, :, c0:c0 + WIN]
                    nc.sync.dma_start(raw[:], sl.rearrange("h d w -> d h w"))
                    nc.sync.dma_start(rot[0:64], sl[:, 64:128, :].rearrange("h d w -> d h w"))
                    nc.sync.dma_start(rot[64:128], sl[:, 0:64, :].rearrange("h d w -> d h w"))
                    cos4 = tp.tile([128, 4, WIN], f32, name="cos4")
                    sin4 = tp.tile([128, 4, WIN], f32, name="sin4")
                    for g in range(4):
                        nc.sync.dma_start(cos4[:, g], COS[:, pos0:pos0 + WIN])
                        nc.sync.dma_start(sin4[:, g], SINS[:, pos0:pos0 + WIN])
                    for g in range(4):
                        gs = slice(g * 4, (g + 1) * 4)
                        tmp = tp.tile([128, 4, WIN], bf16, name="tmp")
                        nc.vector.tensor_mul(tmp[:], rot[:, gs], sin4[:])
                        nc.vector.tensor_mul(roped[:, gs], raw[:, gs], cos4[:])
                        nc.vector.tensor_add(roped[:, gs], roped[:, gs], tmp[:])

                kt_prev = qk.tile([128, HEADS, WIN], bf16, name="kt")
                rope_load(KTS, 0, kt_prev, "k")
                v_prev = qk.tile([128, 2, DIMS], bf16, name="v")
                nc.sync.dma_start(
                    v_prev[:], VS[0:WIN].rearrange("(tb p) c -> p tb c", p=128))

                for c in range(NC_):
                    kt_cur = qk.tile([128, HEADS, WIN], bf16, name="kt")
                    rope_load(KTS, WIN + c * WIN, kt_cur, "k")
                    v_cur = qk.tile([128, 2, DIMS], bf16, name="v")
                    nc.sync.dma_start(
                        v_cur[:], VS[WIN + c * WIN:WIN + (c + 1) * WIN]
                        .rearrange("(tb p) c -> p tb c", p=128))
                    qt = qk.tile([128, HEADS, WIN], bf16, name="qt")
                    rope_load(QTS, c * WIN, qt, "q", pos0=WIN + c * WIN)

                    kts = [kt_prev, kt_prev, kt_cur, kt_cur]
                    vs = [v_prev, v_prev, v_cur, v_cur]
                    W2 = 2 * WIN
                    for h0 in range(0, HEADS, 2):
                        # per head-pair psums: denominator and O^T share
                        # [128, 512] banks (head h0 left, h0+1 right)
                        pd = ps_d.tile([128, W2], f32, name="pd")
                        po = ps_o.tile([128, W2], f32, name="po")
                        pts2 = []
                        for h in (h0, h0 + 1):
                            # scores: kb0|kb1 pair and kb2|kb3 pair in one bank
                            pts = []
                            for pr in range(2):
                                ps = ps_s.tile([128, W2], f32, name="ps")
                                for kb2 in range(2):
                                    kb = pr * 2 + kb2
                                    nc.tensor.matmul(
                                        ps[:, kb2 * WIN:(kb2 + 1) * WIN],
                                        kts[kb][:, h, (kb % 2) * 128:(kb % 2) * 128 + 128],
                                        qt[:, h], start=True, stop=True)
                                pb = ptp.tile([128, W2], bf16, name=f"pt{pr}")
                                if pr == 0:
                                    if c == 0:
                                        nc.scalar.activation(pb[:], ps[:], AF.Exp,
                                                             bias=pgate[:], scale=ISQ)
                                    else:
                                        nc.scalar.activation(pb[:], ps[:], AF.Exp,
                                                             scale=ISQ)
                                else:
                                    ex = exp_pool.tile([128, W2], bf16, name="ex")
                                    nc.scalar.activation(ex[:], ps[:], AF.Exp,
                                                         scale=ISQ)
                                    nc.vector.tensor_mul(pb[:], ex[:], tri23[:])
                                pts.append(pb)
                            pts2.append(pts)

                        for i, h in enumerate((h0, h0 + 1)):
                            sl = slice(i * WIN, (i + 1) * WIN)
                            for kb in range(4):
                                pb = pts2[i][kb // 2][:, (kb % 2) * WIN:(kb % 2 + 1) * WIN]
                                nc.tensor.matmul(pd[:, sl], onesm[:], pb,
                                                 start=(kb == 0), stop=(kb == 3))
                            for kb in range(4):
                                pb = pts2[i][kb // 2][:, (kb % 2) * WIN:(kb % 2 + 1) * WIN]
                                nc.tensor.matmul(
                                    po[:, sl], vs[kb][:, kb % 2, h * 128:(h + 1) * 128],
                                    pb, start=(kb == 0), stop=(kb == 3))
                        rb = obp.tile([128, W2], f32, name="rb")
                        with nc.allow_low_precision("softmax denominator"):
                            nc.vector.reciprocal(rb[:], pd[:])
                        ot = obp.tile([128, W2], f32r, name="ot")
                        nc.vector.tensor_mul(ot[:], po[:], rb[:])
                        nc.sync.dma_start(OTS[h0][:, c * WIN:(c + 1) * WIN],
                                          ot[:, 0:WIN])
                        nc.sync.dma_start(OTS[h0 + 1][:, c * WIN:(c + 1) * WIN],
                                          ot[:, WIN:W2])
                    kt_prev, v_prev = kt_cur, v_cur

            # ---------------- P3: output projection ----------------
            if 3 in phases:
              with tc.tile_pool(name="p3", bufs=1) as p3, \
                 tc.tile_pool(name="otp", bufs=3) as otp, \
                 tc.tile_pool(name="so3", bufs=6) as so3, \
                 tc.tile_pool(name="pp3", bufs=8, space="PSUM") as pp3:
                wo = p3.tile([128, IB, DIMS], f32r)
                nc.sync.dma_start(wo[:], WO.rearrange("(ib p) d -> p ib d", p=128))
                for tt in range(T // 128):
                    ots = otp.tile([128, HEADS, 128], f32r, name="ots")
                    nc.sync.dma_start(
                        ots[:], OTS[:, :, tt * 128:(tt + 1) * 128]
                        .rearrange("h d w -> d h w"))
                    for nt in range(4):
                        ps = pp3.tile([128, 512], f32, name="pp3")
                        for h in range(HEADS):
                            nc.tensor.matmul(
                                ps[:], ots[:, h], wo[:, h, nt * 512:(nt + 1) * 512],
                                start=(h == 0), stop=(h == HEADS - 1))
                        so = so3.tile([128, 512], f32, name="so")
                        nc.scalar.copy(so[:], ps[:])
                        nc.sync.dma_start(
                            OUT[tt * 128:(tt + 1) * 128,
                                nt * 512:(nt + 1) * 512], so[:])
    return nc


def _host_inputs(hidden_states, Wq, Wk, Wv, Wo, T):
    """Build the 8 per-core input maps."""
    TH = T + WIN
    inv_freq = 1.0 / (THETA ** (np.arange(0, HD, 2, dtype=np.float32) / HD))

    qq = np.arange(WIN)[None, :]
    kk = np.arange(128)[:, None]
    tri23 = np.concatenate([(qq >= kk), (qq >= kk + 128)], 1).astype(ml_dtypes.bfloat16)
    onesm_bf = np.ones((128, 128), ml_dtypes.bfloat16)

    Wq, Wk, Wv, Wo = (np.ascontiguousarray(w, np.float32) for w in (Wq, Wk, Wv, Wo))
    in_maps = []
    for core in range(8):
        b, sh = divmod(core, NSH)
        t0 = sh * T
        hs = np.zeros((TH, DIMS), np.float32)
        lo = max(0, t0 - WIN)
        hs[WIN - (t0 - lo):] = hidden_states[b, lo:t0 + T]
        hT = np.ascontiguousarray(hs.T)

        pos = np.arange(t0 - WIN, t0 + T, dtype=np.float32)
        f = np.outer(inv_freq, pos)                      # [64, TH]
        cos = np.concatenate([np.cos(f), np.cos(f)], 0)  # [128, TH]
        sin = np.sin(f)
        sins = np.concatenate([-sin, sin], 0)
        pg = np.full((128, 1), -1e30 if sh == 0 else 0.0, np.float32)
        in_maps.append({
            "HT": hT, "WQ": Wq, "WK": Wk, "WV": Wv, "WO": Wo,
            "COS": cos.astype(np.float32), "SINS": sins.astype(np.float32),
            "TRI23": tri23, "PGATE": pg, "ONESM": onesm_bf,
        })
    return in_maps


_CACHE = {}


def run(hidden_states, Wq, Wk, Wv, Wo, T=S // NSH, **spmd_kwargs):
    key = T
    if key not in _CACHE:
        nc = bacc.Bacc(None)
        build(nc, T)
        nc.finalize()
        _CACHE[key] = nc
    nc = _CACHE[key]
    in_maps = _host_inputs(hidden_states, Wq, Wk, Wv, Wo, T)
    res = run_bass_kernel_spmd(nc, in_maps, core_ids=list(range(8)), **spmd_kwargs)
    outs = [res.results[i]["OUT"] for i in range(8)]
    full = np.empty((B, NSH * T, DIMS), np.float32)
    for core in range(8):
        b, sh = divmod(core, NSH)
        full[b, sh * T:(sh + 1) * T] = outs[core]
    return full, res


def kernel(hidden_states, Wq, Wk, Wv, Wo):
    out, _ = run(np.asarray(hidden_states), Wq, Wk, Wv, Wo)
    return out



# revision 5
# speedup vs baseline: 1.3763x; 1.3763x over previous
"""Block sliding-window attention on 8 TRN2 NeuronCores.

Sharding: sequence-parallel. 8 shards = (batch b in {0,1}) x (quarter s in
0..3); each core owns 2048 consecutive tokens of one batch plus a 256-token
K/V halo from the previous quarter (zeros + -inf gate for the first quarter).
No collectives: each core computes its tokens' full output rows.

v2 pipeline (vs v1: fp8 DoubleRow projections, RoPE fused into P1,
P2/P3 fused per chunk, no OTS round-trip):
  P1: Q^T/K^T/V via 3-term fp8 e4m3 hi/lo matmuls in DoubleRow perf mode
      (h = h8 + hlo at scale 8, W = W8 + Wlo at scale 256; terms
      h8*W8 + h8*Wlo + hlo*W8 share one PSUM at scale 2048; the dropped
      hlo*Wlo term is ~1e-3 relative). RoPE is applied to Q/K right after
      the PSUM drain (rot-half via SBUF->SBUF partition-swap DMA, cos/sin
      resident bf16), roped heads stored to DRAM scratch. V drains to DRAM
      natural-layout scratch.
  P2+P3 fused per 256-token chunk: scores S^T = K Q^T per 128-key block
      (skipping the fully-masked kb3 x first-half-queries block), exp on
      ACT (-1e30 bias gates the no-previous case), 0/1 triangular mask on
      DVE, denominator via all-ones matmul, O^T = V^T P^T, normalize
      (reciprocal on DVE, multiply on Pool engine), then immediately
      out += O_h @ Wo_h accumulated over 16 heads in PSUM - O^T never
      leaves SBUF.
"""
import sys

try:
    import concourse  # noqa: F401
except ImportError:
    sys.path.insert(0, '/opt/trn_rl_repo')

import ml_dtypes
import numpy as np

import concourse.bacc as bacc
import concourse.mybir as mybir
import concourse.tile as tile
from concourse.bass_utils import run_bass_kernel_spmd

f32 = mybir.dt.float32
f32r = mybir.dt.float32r
bf16 = mybir.dt.bfloat16
f8 = mybir.dt.float8e4
AF = mybir.ActivationFunctionType
DR = mybir.MatmulPerfMode.DoubleRow

DIMS = 2048
HEADS = 16
HD = 128           # head dim
WIN = 256          # window / chunk
B, S = 2, 8192
NSH = 4            # seq shards per batch
THETA = 10000.0
ISQ = float(1.0 / np.sqrt(HD))
KP = DIMS // 256   # 8 contraction k-pairs (256 rows each) for DoubleRow
SH_H = 8.0         # fp8 scale for hidden
SH_W = 256.0       # fp8 scale for weights
DESC = 1.0 / (SH_H * SH_W)


def tok_tiles(n, w=512):
    out, a = [], 0
    while a < n:
        out.append((a, min(w, n - a)))
        a += w
    return out


def build(nc, T, phases=(1, 2)):
    """Emit the per-core program. T = local tokens (multiple of 512)."""
    TH = T + WIN                      # with halo
    NC_ = T // WIN                    # chunks
    H8 = nc.dram_tensor("H8", [DIMS, TH], f8, kind="ExternalInput")
    HLO = nc.dram_tensor("HLO", [DIMS, TH], f8, kind="ExternalInput")
    WT8 = {}
    WTLO = {}
    for w_ in ("Q", "K", "V"):
        WT8[w_] = nc.dram_tensor(f"W{w_}8", [KP * 128, 2, DIMS], f8,
                                 kind="ExternalInput")
        WTLO[w_] = nc.dram_tensor(f"W{w_}LO", [KP * 128, 2, DIMS], f8,
                                  kind="ExternalInput")
    WOB = nc.dram_tensor("WOB", [HEADS * 128, DIMS], bf16, kind="ExternalInput")
    COS = nc.dram_tensor("COS", [HD, TH], bf16, kind="ExternalInput")
    SINS = nc.dram_tensor("SINS", [HD, TH], bf16, kind="ExternalInput")
    TRI = nc.dram_tensor("TRI", [128, 384], bf16, kind="ExternalInput")
    PGATE = nc.dram_tensor("PGATE", [128, 1], f32, kind="ExternalInput")
    ONESM = nc.dram_tensor("ONESM", [128, 128], bf16, kind="ExternalInput")
    OUT = nc.dram_tensor("OUT", [T, DIMS], bf16, kind="ExternalOutput")

    KTS = nc.dram_tensor("KTS", [HEADS, HD, TH], bf16)   # roped K^T
    QTS = nc.dram_tensor("QTS", [HEADS, HD, T], bf16)    # roped Q^T
    VS = nc.dram_tensor("VS", [TH, DIMS], bf16)          # V natural

    with tile.TileContext(nc) as tc:
        with tc.tile_pool(name="cst", bufs=1) as cst:
            tri = cst.tile([128, 384], bf16)
            pgate = cst.tile([128, 1], f32)
            onesm = cst.tile([128, 128], bf16)
            nc.sync.dma_start(tri[:], TRI[:])
            nc.sync.dma_start(pgate[:], PGATE[:])
            nc.sync.dma_start(onesm[:], ONESM[:])

            # ---------------- P1: projections + RoPE ----------------
            if 1 in phases:
              with tc.tile_pool(name="hp", bufs=1) as hp, \
                 tc.tile_pool(name="rope_c", bufs=1) as rcp, \
                 tc.tile_pool(name="wp", bufs=1) as wp, \
                 tc.tile_pool(name="dr", bufs=2) as dr, \
                 tc.tile_pool(name="vst", bufs=2) as vst, \
                 tc.tile_pool(name="pp", bufs=6, space="PSUM") as pp:
                h8t, hlot = [], []
                for kp in range(KP):
                    a = hp.tile([128, 2, TH], f8, name=f"h8_{kp}")
                    b_ = hp.tile([128, 2, TH], f8, name=f"hlo_{kp}")
                    sl = slice(kp * 256, (kp + 1) * 256)
                    nc.sync.dma_start(
                        a[:], H8[sl].rearrange("(j p) t -> p j t", p=128))
                    nc.sync.dma_start(
                        b_[:], HLO[sl].rearrange("(j p) t -> p j t", p=128))
                    h8t.append(a)
                    hlot.append(b_)
                cosb = rcp.tile([128, TH], bf16)
                sinb = rcp.tile([128, TH], bf16)
                nc.sync.dma_start(cosb[:], COS[:])
                nc.sync.dma_start(sinb[:], SINS[:])

                def load_w(which):
                    w8, wlo = [], []
                    for kp in range(KP):
                        a = wp.tile([128, 2, DIMS], f8, name=f"w8_{kp}")
                        b_ = wp.tile([128, 2, DIMS], f8, name=f"wlo_{kp}")
                        sl = slice(kp * 128, (kp + 1) * 128)
                        nc.sync.dma_start(a[:], WT8[which][:, :, :]
                                          .rearrange("(kp p) j c -> kp p j c",
                                                     p=128)[kp])
                        nc.sync.dma_start(b_[:], WTLO[which][:, :, :]
                                          .rearrange("(kp p) j c -> kp p j c",
                                                     p=128)[kp])
                        w8.append(a)
                        wlo.append(b_)
                    return w8, wlo

                def mm3(ps, lhs_pairs, rhs_pairs, kp, npairs):
                    """3-term fp8 DoubleRow accumulate into psum."""
                    (l8, llo), (r8, rlo) = lhs_pairs, rhs_pairs
                    nc.tensor.matmul(ps, l8, r8, start=(kp == 0), stop=False,
                                     perf_mode=DR)
                    nc.tensor.matmul(ps, l8, rlo, start=False, stop=False,
                                     perf_mode=DR)
                    nc.tensor.matmul(ps, llo, r8, start=False,
                                     stop=(kp == npairs - 1), perf_mode=DR)

                # K then Q (head-transposed layout + RoPE), V natural last
                for which, DST, t0, tlen in (("K", KTS, 0, TH),
                                             ("Q", QTS, WIN, T)):
                    w8, wlo = load_w(which)
                    for ob in range(HEADS):
                        hb = dr.tile([128, TH], bf16, name="hb")
                        for a, w in tok_tiles(tlen):
                            ps = pp.tile([128, 512], f32, name="pp")
                            for kp in range(KP):
                                mm3(ps[:, :w],
                                    (w8[kp][:, :, ob * 128:(ob + 1) * 128],
                                     wlo[kp][:, :, ob * 128:(ob + 1) * 128]),
                                    (h8t[kp][:, :, t0 + a:t0 + a + w],
                                     hlot[kp][:, :, t0 + a:t0 + a + w]),
                                    kp, KP)
                            nc.scalar.mul(hb[:, a:a + w], ps[:, :w], DESC)
                        # RoPE: rot-half via partition-swap SBUF->SBUF DMA
                        rot = dr.tile([128, TH], bf16, name="rot")
                        nc.sync.dma_start(rot[0:64], hb[64:128])
                        nc.sync.dma_start(rot[64:128], hb[0:64])
                        tmp = dr.tile([128, TH], bf16, name="tmp")
                        ro = dr.tile([128, TH], bf16, name="ro")
                        csl = slice(0, TH) if which == "K" else slice(WIN, TH)
                        n = TH if which == "K" else T
                        nc.vector.tensor_mul(tmp[:, :n], rot[:, :n],
                                             sinb[:, csl])
                        nc.vector.tensor_mul(ro[:, :n], hb[:, :n],
                                             cosb[:, csl])
                        nc.vector.tensor_add(ro[:, :n], ro[:, :n], tmp[:, :n])
                        nc.sync.dma_start(DST[ob][:, :], ro[:, :n])

                # V natural: lhsT = h pair [128,2,128tok], rhs = Wv [128,2,512]
                wv8, wvlo = load_w("V")
                for tb in range(TH // 128):
                    tsl = slice(tb * 128, tb * 128 + 128)
                    vsb = vst.tile([128, DIMS], bf16, name="vsb")
                    for og in range(4):
                        ps = pp.tile([128, 512], f32, name="pp")
                        ogs = slice(og * 512, (og + 1) * 512)
                        for kp in range(KP):
                            mm3(ps[:], (h8t[kp][:, :, tsl],
                                        hlot[kp][:, :, tsl]),
                                (wv8[kp][:, :, ogs], wvlo[kp][:, :, ogs]),
                                kp, KP)
                        nc.scalar.mul(vsb[:, ogs], ps[:], DESC)
                    nc.sync.dma_start(VS[tb * 128:(tb + 1) * 128, :], vsb[:])

            # ---------------- P2+P3 fused per chunk ----------------
            if 2 in phases:
              with tc.tile_pool(name="wo", bufs=1) as wop, \
                 tc.tile_pool(name="kq", bufs=3) as kq, \
                 tc.tile_pool(name="pb", bufs=6) as pbp, \
                 tc.tile_pool(name="ot", bufs=10) as otp, \
                 tc.tile_pool(name="rb", bufs=2) as rbp, \
                 tc.tile_pool(name="ou", bufs=2) as oup, \
                 tc.tile_pool(name="ps_s", bufs=2, space="PSUM") as ps_s, \
                 tc.tile_pool(name="ps_d", bufs=2, space="PSUM") as ps_d, \
                 tc.tile_pool(name="ps_b", bufs=2, space="PSUM") as ps_b:
                wo = wop.tile([128, HEADS, DIMS], bf16)
                nc.sync.dma_start(wo[:], WOB.rearrange("(h p) d -> p h d",
                                                       p=128))

                def load_kt(c0):
                    t = kq.tile([128, HEADS, WIN], bf16, name="kt")
                    nc.sync.dma_start(t[:], KTS[:, :, c0:c0 + WIN]
                                      .rearrange("h d w -> d h w"))
                    return t

                kt_prev = load_kt(0)
                v_prev = kq.tile([128, 2, DIMS], bf16, name="v")
                nc.sync.dma_start(
                    v_prev[:], VS[0:WIN].rearrange("(tb p) c -> p tb c", p=128))

                for c in range(NC_):
                    kt_cur = load_kt(WIN + c * WIN)
                    v_cur = kq.tile([128, 2, DIMS], bf16, name="v")
                    nc.sync.dma_start(
                        v_cur[:], VS[WIN + c * WIN:WIN + (c + 1) * WIN]
                        .rearrange("(tb p) c -> p tb c", p=128))
                    qt = kq.tile([128, HEADS, WIN], bf16, name="qt")
                    nc.sync.dma_start(qt[:], QTS[:, :, c * WIN:(c + 1) * WIN]
                                      .rearrange("h d w -> d h w"))

                    kts = [kt_prev, kt_prev, kt_cur, kt_cur]
                    vs = [v_prev, v_prev, v_cur, v_cur]
                    ots = []
                    for h0 in range(0, HEADS, 2):
                        pd = ps_d.tile([128, 512], f32, name="pd")
                        po = ps_b.tile([128, 512], f32, name="po")
                        pbs2 = []
                        for i, h in enumerate((h0, h0 + 1)):
                            # scores: pb0 = P(kb0|kb1) [128,512],
                            # pb1 = P(kb2 q0:256 | kb3 q128:256) [128,384]
                            ps0 = ps_s.tile([128, 512], f32, name="ps0")
                            for kb in range(2):
                                nc.tensor.matmul(
                                    ps0[:, kb * WIN:(kb + 1) * WIN],
                                    kts[kb][:, h, kb * 128:kb * 128 + 128],
                                    qt[:, h], start=True, stop=True)
                            ps1 = ps_s.tile([128, 512], f32, name="ps1")
                            nc.tensor.matmul(
                                ps1[:, 0:WIN], kts[2][:, h, 0:128],
                                qt[:, h], start=True, stop=True)
                            nc.tensor.matmul(
                                ps1[:, WIN:WIN + 128], kts[3][:, h, 128:256],
                                qt[:, h, 128:256], start=True, stop=True)
                            pb0 = pbp.tile([128, 512], bf16, name="pb0")
                            if c == 0:
                                nc.scalar.activation(pb0[:], ps0[:], AF.Exp,
                                                     bias=pgate[:], scale=ISQ)
                            else:
                                nc.scalar.activation(pb0[:], ps0[:], AF.Exp,
                                                     scale=ISQ)
                            eb1 = pbp.tile([128, 384], bf16, name="eb1")
                            nc.scalar.activation(eb1[:], ps1[:, 0:384], AF.Exp,
                                                 scale=ISQ)
                            pb1 = pbp.tile([128, 384], bf16, name="pb1")
                            nc.vector.tensor_mul(pb1[:], eb1[:], tri[:])
                            pbs2.append((pb0, pb1))

                        for i, h in enumerate((h0, h0 + 1)):
                            pb0, pb1 = pbs2[i]
                            hc = i * WIN
                            # column-split PSUM groups: q 0:128 gets kb0-2,
                            # q 128:256 gets kb0-3
                            gA = [pb0[:, 0:128], pb0[:, WIN:WIN + 128],
                                  pb1[:, 0:128]]
                            gB = [pb0[:, 128:WIN], pb0[:, WIN + 128:512],
                                  pb1[:, 128:WIN], pb1[:, WIN:384]]
                            vA = [vs[0][:, 0, h * 128:(h + 1) * 128],
                                  vs[1][:, 1, h * 128:(h + 1) * 128],
                                  vs[2][:, 0, h * 128:(h + 1) * 128]]
                            vB = vA + [vs[3][:, 1, h * 128:(h + 1) * 128]]
                            for j, pbx in enumerate(gA):
                                nc.tensor.matmul(
                                    pd[:, hc:hc + 128], onesm[:], pbx,
                                    start=(j == 0), stop=(j == len(gA) - 1))
                            for j, pbx in enumerate(gB):
                                nc.tensor.matmul(
                                    pd[:, hc + 128:hc + WIN], onesm[:], pbx,
                                    start=(j == 0), stop=(j == len(gB) - 1))
                            for j, pbx in enumerate(gA):
                                nc.tensor.matmul(
                                    po[:, hc:hc + 128], vA[j], pbx,
                                    start=(j == 0), stop=(j == len(gA) - 1))
                            for j, pbx in enumerate(gB):
                                nc.tensor.matmul(
                                    po[:, hc + 128:hc + WIN], vB[j], pbx,
                                    start=(j == 0), stop=(j == len(gB) - 1))
                        rb = rbp.tile([128, 512], f32, name="rb")
                        with nc.allow_low_precision("softmax denominator"):
                            nc.vector.reciprocal(rb[:], pd[:])
                        ot = otp.tile([128, 512], bf16, name="ot")
                        nc.vector.tensor_mul(ot[:], po[:], rb[:])
                        ots.append(ot)

                    # P3 for this chunk: out[tt] = sum_h O_h @ Wo_h
                    for tt in range(2):
                        ob_ = oup.tile([128, DIMS], bf16, name="ob")
                        for nt in range(4):
                            ps3 = ps_b.tile([128, 512], f32, name="po")
                            for h in range(HEADS):
                                nc.tensor.matmul(
                                    ps3[:],
                                    ots[h // 2][:, (h % 2) * WIN + tt * 128:
                                                (h % 2) * WIN + tt * 128 + 128],
                                    wo[:, h, nt * 512:(nt + 1) * 512],
                                    start=(h == 0), stop=(h == HEADS - 1))
                            nc.scalar.copy(ob_[:, nt * 512:(nt + 1) * 512],
                                           ps3[:])
                        nc.sync.dma_start(
                            OUT[c * WIN + tt * 128:c * WIN + (tt + 1) * 128, :],
                            ob_[:])
                    kt_prev, v_prev = kt_cur, v_cur
    return nc


def _q8(x, s):
    """Quantize x*s to fp8 e4m3; returns (fp8_array, residual_fp8_array)."""
    hi = (x * s).astype(ml_dtypes.float8_e4m3)
    lo = (x * s - hi.astype(np.float32)).astype(ml_dtypes.float8_e4m3)
    return hi, lo


def _host_inputs(hidden_states, Wq, Wk, Wv, Wo, T):
    """Build the 8 per-core input maps."""
    TH = T + WIN
    inv_freq = 1.0 / (THETA ** (np.arange(0, HD, 2, dtype=np.float32) / HD))

    qq = np.arange(WIN)[None, :]
    kk = np.arange(128)[:, None]
    tri = np.concatenate([(qq >= kk),
                          (qq[:, :128] >= kk)], 1).astype(ml_dtypes.bfloat16)
    onesm_bf = np.ones((128, 128), ml_dtypes.bfloat16)

    # weights: fp8 hi/lo pairs in DoubleRow layout [KP*128, 2, DIMS]
    wts = {}
    for name, W in (("Q", Wq), ("K", Wk), ("V", Wv)):
        W = np.ascontiguousarray(W, np.float32)
        hi, lo = _q8(W, SH_W)
        # row r = (kp*2 + j)*128 + p  ->  layout [kp, p, j, c] -> [(kp p), j, c]
        hi = np.ascontiguousarray(
            hi.reshape(KP, 2, 128, DIMS).transpose(0, 2, 1, 3)
            .reshape(KP * 128, 2, DIMS))
        lo = np.ascontiguousarray(
            lo.reshape(KP, 2, 128, DIMS).transpose(0, 2, 1, 3)
            .reshape(KP * 128, 2, DIMS))
        wts[f"W{name}8"] = hi
        wts[f"W{name}LO"] = lo
    wob = np.ascontiguousarray(Wo, np.float32).astype(ml_dtypes.bfloat16)

    in_maps = []
    for core in range(8):
        b, sh = divmod(core, NSH)
        t0 = sh * T
        hs = np.zeros((TH, DIMS), np.float32)
        lo_t = max(0, t0 - WIN)
        hs[WIN - (t0 - lo_t):] = hidden_states[b, lo_t:t0 + T]
        hT = np.ascontiguousarray(hs.T)
        h8, hlo = _q8(hT, SH_H)

        pos = np.arange(t0 - WIN, t0 + T, dtype=np.float32)
        f = np.outer(inv_freq, pos)                      # [64, TH]
        cos = np.concatenate([np.cos(f), np.cos(f)], 0)  # [128, TH]
        sin = np.sin(f)
        sins = np.concatenate([-sin, sin], 0)
        pg = np.full((128, 1), -1e30 if sh == 0 else 0.0, np.float32)
        in_maps.append({
            "H8": h8, "HLO": hlo, **wts, "WOB": wob,
            "COS": cos.astype(ml_dtypes.bfloat16),
            "SINS": sins.astype(ml_dtypes.bfloat16),
            "TRI": tri, "PGATE": pg, "ONESM": onesm_bf,
        })
    return in_maps


_CACHE = {}


def run(hidden_states, Wq, Wk, Wv, Wo, T=S // NSH, **spmd_kwargs):
    key = T
    if key not in _CACHE:
        nc = bacc.Bacc(None)
        build(nc, T)
        nc.finalize()
        _CACHE[key] = nc
    nc = _CACHE[key]
    in_maps = _host_inputs(hidden_states, Wq, Wk, Wv, Wo, T)
    res = run_bass_kernel_spmd(nc, in_maps, core_ids=list(range(8)), **spmd_kwargs)
    outs = [res.results[i]["OUT"] for i in range(8)]
    full = np.empty((B, NSH * T, DIMS), np.float32)
    for core in range(8):
        b, sh = divmod(core, NSH)
        full[b, sh * T:(sh + 1) * T] = outs[core].astype(np.float32)
    return full, res


def kernel(hidden_states, Wq, Wk, Wv, Wo):
    out, _ = run(np.asarray(hidden_states), Wq, Wk, Wv, Wo)
    return out


# revision 9
# speedup vs baseline: 1.4285x; 1.0379x over previous
"""Block sliding-window attention on 8 TRN2 NeuronCores.

Sharding: sequence-parallel. 8 shards = (batch b in {0,1}) x (quarter s in
0..3); each core owns 2048 consecutive tokens of one batch plus a 256-token
K/V halo from the previous quarter (zeros + -inf gate for the first quarter).
No collectives: each core computes its tokens' full output rows.

v2 pipeline (vs v1: fp8 DoubleRow projections, RoPE fused into P1,
P2/P3 fused per chunk, no OTS round-trip):
  P1: Q^T/K^T/V via 3-term fp8 e4m3 hi/lo matmuls in DoubleRow perf mode
      (h = h8 + hlo at scale 8, W = W8 + Wlo at scale 256; terms
      h8*W8 + h8*Wlo + hlo*W8 share one PSUM at scale 2048; the dropped
      hlo*Wlo term is ~1e-3 relative). RoPE is applied to Q/K right after
      the PSUM drain (rot-half via SBUF->SBUF partition-swap DMA, cos/sin
      resident bf16), roped heads stored to DRAM scratch. V drains to DRAM
      natural-layout scratch.
  P2+P3 fused per 256-token chunk: scores S^T = K Q^T per 128-key block
      (skipping the fully-masked kb3 x first-half-queries block), exp on
      ACT (-1e30 bias gates the no-previous case), 0/1 triangular mask on
      DVE, denominator via all-ones matmul, O^T = V^T P^T, normalize
      (reciprocal on DVE, multiply on Pool engine), then immediately
      out += O_h @ Wo_h accumulated over 16 heads in PSUM - O^T never
      leaves SBUF.
"""
import sys

try:
    import concourse  # noqa: F401
except ImportError:
    sys.path.insert(0, '/opt/trn_rl_repo')

import ml_dtypes
import numpy as np

import concourse.bacc as bacc
import concourse.mybir as mybir
import concourse.tile as tile
from concourse.bass_utils import run_bass_kernel_spmd

f32 = mybir.dt.float32
f32r = mybir.dt.float32r
bf16 = mybir.dt.bfloat16
f8 = mybir.dt.float8e4
AF = mybir.ActivationFunctionType
DR = mybir.MatmulPerfMode.DoubleRow

DIMS = 2048
HEADS = 16
HD = 128           # head dim
WIN = 256          # window / chunk
B, S = 2, 8192
NSH = 4            # seq shards per batch
THETA = 10000.0
ISQ = float(1.0 / np.sqrt(HD))
KP = DIMS // 256   # 8 contraction k-pairs (256 rows each) for DoubleRow
SH_H = 8.0         # fp8 scale for hidden
SH_W = 256.0       # fp8 scale for weights
DESC = 1.0 / (SH_H * SH_W)


def tok_tiles(n, w=512):
    out, a = [], 0
    while a < n:
        out.append((a, min(w, n - a)))
        a += w
    return out


def build(nc, T, phases=(1, 2)):
    """Emit the per-core program. T = local tokens (multiple of 512)."""
    TH = T + WIN                      # with halo
    NC_ = T // WIN                    # chunks
    H8 = nc.dram_tensor("H8", [DIMS, TH], f8, kind="ExternalInput")
    HLO = nc.dram_tensor("HLO", [DIMS, TH], f8, kind="ExternalInput")
    WT8 = {}
    WTLO = {}
    for w_ in ("Q", "K", "V"):
        WT8[w_] = nc.dram_tensor(f"W{w_}8", [KP * 128, 2, DIMS], f8,
                                 kind="ExternalInput")
        WTLO[w_] = nc.dram_tensor(f"W{w_}LO", [KP * 128, 2, DIMS], f8,
                                  kind="ExternalInput")
    WOB = nc.dram_tensor("WOB", [HEADS * 128, DIMS], bf16, kind="ExternalInput")
    COS = nc.dram_tensor("COS", [HD, TH], bf16, kind="ExternalInput")
    SINS = nc.dram_tensor("SINS", [HD, TH], bf16, kind="ExternalInput")
    TRI = nc.dram_tensor("TRI", [128, 384], bf16, kind="ExternalInput")
    PGATE = nc.dram_tensor("PGATE", [128, 1], f32, kind="ExternalInput")
    ONESM = nc.dram_tensor("ONESM", [128, 128], bf16, kind="ExternalInput")
    OUT = nc.dram_tensor("OUT", [T, DIMS], bf16, kind="ExternalOutput")

    KTS = nc.dram_tensor("KTS", [HEADS, HD, TH], bf16)   # roped K^T
    QTS = nc.dram_tensor("QTS", [HEADS, HD, T], bf16)    # roped Q^T
    VS = nc.dram_tensor("VS", [TH, DIMS], bf16)          # V natural

    with tile.TileContext(nc) as tc:
        with tc.tile_pool(name="cst", bufs=1) as cst:
            tri = cst.tile([128, 384], bf16)
            pgate = cst.tile([128, 1], f32)
            onesm = cst.tile([128, 128], bf16)
            nc.sync.dma_start(tri[:], TRI[:])
            nc.sync.dma_start(pgate[:], PGATE[:])
            nc.sync.dma_start(onesm[:], ONESM[:])

            # ---------------- P1: projections + RoPE ----------------
            if 1 in phases:
              with tc.tile_pool(name="hp", bufs=1) as hp, \
                 tc.tile_pool(name="rope_c", bufs=1) as rcp, \
                 tc.tile_pool(name="wp", bufs=1) as wp, \
                 tc.tile_pool(name="dr", bufs=2) as dr, \
                 tc.tile_pool(name="vst", bufs=2) as vst, \
                 tc.tile_pool(name="pp", bufs=6, space="PSUM") as pp:
                def load_w_half(which, half):
                    """Load output-column half of a weight pair (dbl-buffered)."""
                    w8, wlo = [], []
                    csl = slice(half * DIMS // 2, (half + 1) * DIMS // 2)
                    for kp in range(KP):
                        a = wp.tile([128, 2, DIMS // 2], f8, name=f"w8_{kp}")
                        b_ = wp.tile([128, 2, DIMS // 2], f8, name=f"wlo_{kp}")
                        nc.sync.dma_start(a[:], WT8[which]
                                          .rearrange("(kp p) j c -> kp p j c",
                                                     p=128)[kp][:, :, csl])
                        nc.sync.dma_start(b_[:], WTLO[which]
                                          .rearrange("(kp p) j c -> kp p j c",
                                                     p=128)[kp][:, :, csl])
                        w8.append(a)
                        wlo.append(b_)
                    return w8, wlo

                # startup: interleave K-weight half 0 with the hidden loads
                wk_halves = [load_w_half("K", 0)]
                h8t, hlot = [], []
                for kp in range(KP):
                    a = hp.tile([128, 2, TH], f8, name=f"h8_{kp}")
                    b_ = hp.tile([128, 2, TH], f8, name=f"hlo_{kp}")
                    sl = slice(kp * 256, (kp + 1) * 256)
                    nc.sync.dma_start(
                        a[:], H8[sl].rearrange("(j p) t -> p j t", p=128))
                    nc.sync.dma_start(
                        b_[:], HLO[sl].rearrange("(j p) t -> p j t", p=128))
                    h8t.append(a)
                    hlot.append(b_)
                cosb = rcp.tile([128, TH], bf16)
                sinb = rcp.tile([128, TH], bf16)
                nc.sync.dma_start(cosb[:], COS[:])
                nc.sync.dma_start(sinb[:], SINS[:])

                def mm3(ps, lhs_pairs, rhs_pairs, kp, npairs):
                    """3-term fp8 DoubleRow accumulate into psum."""
                    (l8, llo), (r8, rlo) = lhs_pairs, rhs_pairs
                    nc.tensor.matmul(ps, l8, r8, start=(kp == 0), stop=False,
                                     perf_mode=DR)
                    nc.tensor.matmul(ps, l8, rlo, start=False, stop=False,
                                     perf_mode=DR)
                    nc.tensor.matmul(ps, llo, r8, start=False,
                                     stop=(kp == npairs - 1), perf_mode=DR)

                # K then Q (head-transposed layout + RoPE), V natural last.
                # Weights stream in output-column halves, double-buffered:
                # while obs of one half run, the next half (or matrix) loads.
                w_seq = [("K", 1), ("Q", 0), ("Q", 1), ("V", 0), ("V", 1)]
                for which, DST, t0, tlen in (("K", KTS, 0, TH),
                                             ("Q", QTS, WIN, T)):
                    for half in range(2):
                        w8, wlo = wk_halves.pop(0)
                        if w_seq:
                            wk_halves.append(load_w_half(*w_seq.pop(0)))
                        for ob in range(half * 8, half * 8 + 8):
                            oc = (ob % 8) * 128
                            hb = dr.tile([128, TH], bf16, name="hb")
                            for a, w in tok_tiles(tlen):
                                ps = pp.tile([128, 512], f32, name="pp")
                                for kp in range(KP):
                                    mm3(ps[:, :w],
                                        (w8[kp][:, :, oc:oc + 128],
                                         wlo[kp][:, :, oc:oc + 128]),
                                        (h8t[kp][:, :, t0 + a:t0 + a + w],
                                         hlot[kp][:, :, t0 + a:t0 + a + w]),
                                        kp, KP)
                                nc.scalar.mul(hb[:, a:a + w], ps[:, :w], DESC)
                            # RoPE: rot-half via partition-swap SBUF->SBUF DMA
                            rot = dr.tile([128, TH], bf16, name="rot")
                            nc.sync.dma_start(rot[0:64], hb[64:128])
                            nc.sync.dma_start(rot[64:128], hb[0:64])
                            tmp = dr.tile([128, TH], bf16, name="tmp")
                            ro = dr.tile([128, TH], bf16, name="ro")
                            csl = slice(0, TH) if which == "K" else slice(WIN, TH)
                            n = TH if which == "K" else T
                            nc.vector.tensor_mul(tmp[:, :n], rot[:, :n],
                                                 sinb[:, csl])
                            nc.vector.tensor_mul(ro[:, :n], hb[:, :n],
                                                 cosb[:, csl])
                            nc.vector.tensor_add(ro[:, :n], ro[:, :n],
                                                 tmp[:, :n])
                            nc.sync.dma_start(DST[ob][:, :], ro[:, :n])

                # V natural: lhsT = h pair [128,2,128tok], rhs = Wv [128,2,512]
                for half in range(2):
                    wv8, wvlo = wk_halves.pop(0)
                    if w_seq:
                        wk_halves.append(load_w_half(*w_seq.pop(0)))
                    for tb in range(TH // 128):
                        tsl = slice(tb * 128, tb * 128 + 128)
                        vsb = vst.tile([128, DIMS // 2], bf16, name="vsb")
                        for og in range(2):
                            ps = pp.tile([128, 512], f32, name="pp")
                            ogs = slice(og * 512, (og + 1) * 512)
                            for kp in range(KP):
                                mm3(ps[:], (h8t[kp][:, :, tsl],
                                            hlot[kp][:, :, tsl]),
                                    (wv8[kp][:, :, ogs], wvlo[kp][:, :, ogs]),
                                    kp, KP)
                            nc.scalar.mul(vsb[:, ogs], ps[:], DESC)
                        nc.sync.dma_start(
                            VS[tb * 128:(tb + 1) * 128,
                               half * 1024:(half + 1) * 1024], vsb[:])

            # ---------------- P2+P3 fused per chunk ----------------
            if 2 in phases:
              with tc.tile_pool(name="wo", bufs=1) as wop, \
                 tc.tile_pool(name="kq", bufs=3) as kq, \
                 tc.tile_pool(name="pb", bufs=6) as pbp, \
                 tc.tile_pool(name="ot", bufs=10) as otp, \
                 tc.tile_pool(name="rb", bufs=2) as rbp, \
                 tc.tile_pool(name="ou", bufs=2) as oup, \
                 tc.tile_pool(name="ps_s", bufs=2, space="PSUM") as ps_s, \
                 tc.tile_pool(name="ps_d", bufs=2, space="PSUM") as ps_d, \
                 tc.tile_pool(name="ps_b", bufs=2, space="PSUM") as ps_b:
                def load_kt(c0):
                    t = kq.tile([128, HEADS, WIN], bf16, name="kt")
                    nc.sync.dma_start(t[:], KTS[:, :, c0:c0 + WIN]
                                      .rearrange("h d w -> d h w"))
                    return t

                kt_prev = load_kt(0)
                v_prev = kq.tile([128, 2, DIMS], bf16, name="v")
                nc.sync.dma_start(
                    v_prev[:], VS[0:WIN].rearrange("(tb p) c -> p tb c", p=128))
                # wo loads AFTER the first chunk inputs, split in 4 so chunk
                # DMAs can interleave; P2 of early chunks runs before it lands
                wo = wop.tile([128, HEADS, DIMS], bf16)
                wor = WOB.rearrange("(h p) d -> p h d", p=128)
                for hg in range(4):
                    nc.sync.dma_start(wo[:, hg * 4:(hg + 1) * 4],
                                      wor[:, hg * 4:(hg + 1) * 4])

                for c in range(NC_):
                    kt_cur = load_kt(WIN + c * WIN)
                    v_cur = kq.tile([128, 2, DIMS], bf16, name="v")
                    nc.sync.dma_start(
                        v_cur[:], VS[WIN + c * WIN:WIN + (c + 1) * WIN]
                        .rearrange("(tb p) c -> p tb c", p=128))
                    qt = kq.tile([128, HEADS, WIN], bf16, name="qt")
                    nc.sync.dma_start(qt[:], QTS[:, :, c * WIN:(c + 1) * WIN]
                                      .rearrange("h d w -> d h w"))

                    kts = [kt_prev, kt_prev, kt_cur, kt_cur]
                    vs = [v_prev, v_prev, v_cur, v_cur]
                    ots = []
                    for h0 in range(0, HEADS, 2):
                        pd = ps_d.tile([128, 512], f32, name="pd")
                        po = ps_b.tile([128, 512], f32, name="po")
                        pbs2 = []
                        for i, h in enumerate((h0, h0 + 1)):
                            # scores: pb0 = P(kb0|kb1) [128,512],
                            # pb1 = P(kb2 q0:256 | kb3 q128:256) [128,384]
                            ps0 = ps_s.tile([128, 512], f32, name="ps0")
                            for kb in range(2):
                                nc.tensor.matmul(
                                    ps0[:, kb * WIN:(kb + 1) * WIN],
                                    kts[kb][:, h, kb * 128:kb * 128 + 128],
                                    qt[:, h], start=True, stop=True)
                            ps1 = ps_s.tile([128, 512], f32, name="ps1")
                            nc.tensor.matmul(
                                ps1[:, 0:WIN], kts[2][:, h, 0:128],
                                qt[:, h], start=True, stop=True)
                            nc.tensor.matmul(
                                ps1[:, WIN:WIN + 128], kts[3][:, h, 128:256],
                                qt[:, h, 128:256], start=True, stop=True)
                            pb0 = pbp.tile([128, 512], bf16, name="pb0")
                            if c == 0:
                                nc.scalar.activation(pb0[:], ps0[:], AF.Exp,
                                                     bias=pgate[:], scale=ISQ)
                            else:
                                nc.scalar.activation(pb0[:], ps0[:], AF.Exp,
                                                     scale=ISQ)
                            eb1 = pbp.tile([128, 384], bf16, name="eb1")
                            nc.scalar.activation(eb1[:], ps1[:, 0:384], AF.Exp,
                                                 scale=ISQ)
                            pb1 = pbp.tile([128, 384], bf16, name="pb1")
                            nc.vector.tensor_mul(pb1[:], eb1[:], tri[:])
                            pbs2.append((pb0, pb1))

                        for i, h in enumerate((h0, h0 + 1)):
                            pb0, pb1 = pbs2[i]
                            hc = i * WIN
                            # column-split PSUM groups: q 0:128 gets kb0-2,
                            # q 128:256 gets kb0-3
                            gA = [pb0[:, 0:128], pb0[:, WIN:WIN + 128],
                                  pb1[:, 0:128]]
                            gB = [pb0[:, 128:WIN], pb0[:, WIN + 128:512],
                                  pb1[:, 128:WIN], pb1[:, WIN:384]]
                            vA = [vs[0][:, 0, h * 128:(h + 1) * 128],
                                  vs[1][:, 1, h * 128:(h + 1) * 128],
                                  vs[2][:, 0, h * 128:(h + 1) * 128]]
                            vB = vA + [vs[3][:, 1, h * 128:(h + 1) * 128]]
                            for j, pbx in enumerate(gA):
                                nc.tensor.matmul(
                                    pd[:, hc:hc + 128], onesm[:], pbx,
                                    start=(j == 0), stop=(j == len(gA) - 1))
                            for j, pbx in enumerate(gB):
                                nc.tensor.matmul(
                                    pd[:, hc + 128:hc + WIN], onesm[:], pbx,
                                    start=(j == 0), stop=(j == len(gB) - 1))
                            for j, pbx in enumerate(gA):
                                nc.tensor.matmul(
                                    po[:, hc:hc + 128], vA[j], pbx,
                                    start=(j == 0), stop=(j == len(gA) - 1))
                            for j, pbx in enumerate(gB):
                                nc.tensor.matmul(
                                    po[:, hc + 128:hc + WIN], vB[j], pbx,
                                    start=(j == 0), stop=(j == len(gB) - 1))
                        rb = rbp.tile([128, 512], f32, name="rb")
                        with nc.allow_low_precision("softmax denominator"):
                            nc.vector.reciprocal(rb[:], pd[:])
                        ot = otp.tile([128, 512], bf16, name="ot")
                        nc.vector.tensor_mul(ot[:], po[:], rb[:])
                        ots.append(ot)

                    # P3 for this chunk: out[tt] = sum_h O_h @ Wo_h
                    for tt in range(2):
                        ob_ = oup.tile([128, DIMS], bf16, name="ob")
                        for nt in range(4):
                            ps3 = ps_b.tile([128, 512], f32, name="po")
                            for h in range(HEADS):
                                nc.tensor.matmul(
                                    ps3[:],
                                    ots[h // 2][:, (h % 2) * WIN + tt * 128:
                                                (h % 2) * WIN + tt * 128 + 128],
                                    wo[:, h, nt * 512:(nt + 1) * 512],
                                    start=(h == 0), stop=(h == HEADS - 1))
                            nc.scalar.copy(ob_[:, nt * 512:(nt + 1) * 512],
                                           ps3[:])
                        nc.sync.dma_start(
                            OUT[c * WIN + tt * 128:c * WIN + (tt + 1) * 128, :],
                            ob_[:])
                    kt_prev, v_prev = kt_cur, v_cur
    return nc


def _q8(x, s):
    """Quantize x*s to fp8 e4m3; returns (fp8_array, residual_fp8_array)."""
    hi = (x * s).astype(ml_dtypes.float8_e4m3)
    lo = (x * s - hi.astype(np.float32)).astype(ml_dtypes.float8_e4m3)
    return hi, lo


def _host_inputs(hidden_states, Wq, Wk, Wv, Wo, T):
    """Build the 8 per-core input maps."""
    TH = T + WIN
    inv_freq = 1.0 / (THETA ** (np.arange(0, HD, 2, dtype=np.float32) / HD))

    qq = np.arange(WIN)[None, :]
    kk = np.arange(128)[:, None]
    tri = np.concatenate([(qq >= kk),
                          (qq[:, :128] >= kk)], 1).astype(ml_dtypes.bfloat16)
    onesm_bf = np.ones((128, 128), ml_dtypes.bfloat16)

    # weights: fp8 hi/lo pairs in DoubleRow layout [KP*128, 2, DIMS]
    wts = {}
    for name, W in (("Q", Wq), ("K", Wk), ("V", Wv)):
        W = np.ascontiguousarray(W, np.float32)
        hi, lo = _q8(W, SH_W)
        # row r = (kp*2 + j)*128 + p  ->  layout [kp, p, j, c] -> [(kp p), j, c]
        hi = np.ascontiguousarray(
            hi.reshape(KP, 2, 128, DIMS).transpose(0, 2, 1, 3)
            .reshape(KP * 128, 2, DIMS))
        lo = np.ascontiguousarray(
            lo.reshape(KP, 2, 128, DIMS).transpose(0, 2, 1, 3)
            .reshape(KP * 128, 2, DIMS))
        wts[f"W{name}8"] = hi
        wts[f"W{name}LO"] = lo
    wob = np.ascontiguousarray(Wo, np.float32).astype(ml_dtypes.bfloat16)

    in_maps = []
    for core in range(8):
        b, sh = divmod(core, NSH)
        t0 = sh * T
        hs = np.zeros((TH, DIMS), np.float32)
        lo_t = max(0, t0 - WIN)
        hs[WIN - (t0 - lo_t):] = hidden_states[b, lo_t:t0 + T]
        hT = np.ascontiguousarray(hs.T)
        h8, hlo = _q8(hT, SH_H)

        pos = np.arange(t0 - WIN, t0 + T, dtype=np.float32)
        f = np.outer(inv_freq, pos)                      # [64, TH]
        cos = np.concatenate([np.cos(f), np.cos(f)], 0)  # [128, TH]
        sin = np.sin(f)
        sins = np.concatenate([-sin, sin], 0)
        pg = np.full((128, 1), -1e30 if sh == 0 else 0.0, np.float32)
        in_maps.append({
            "H8": h8, "HLO": hlo, **wts, "WOB": wob,
            "COS": cos.astype(ml_dtypes.bfloat16),
            "SINS": sins.astype(ml_dtypes.bfloat16),
            "TRI": tri, "PGATE": pg, "ONESM": onesm_bf,
        })
    return in_maps


_CACHE = {}


def run(hidden_states, Wq, Wk, Wv, Wo, T=S // NSH, **spmd_kwargs):
    key = T
    if key not in _CACHE:
        nc = bacc.Bacc(None)
        build(nc, T)
        nc.finalize()
        _CACHE[key] = nc
    nc = _CACHE[key]
    in_maps = _host_inputs(hidden_states, Wq, Wk, Wv, Wo, T)
    res = run_bass_kernel_spmd(nc, in_maps, core_ids=list(range(8)), **spmd_kwargs)
    outs = [res.results[i]["OUT"] for i in range(8)]
    full = np.empty((B, NSH * T, DIMS), np.float32)
    for core in range(8):
        b, sh = divmod(core, NSH)
        full[b, sh * T:(sh + 1) * T] = outs[core].astype(np.float32)
    return full, res


def kernel(hidden_states, Wq, Wk, Wv, Wo):
    out, _ = run(np.asarray(hidden_states), Wq, Wk, Wv, Wo)
    return out
